# revision 4
# baseline (speedup 1.0000x reference)
"""Trainium2 Bass kernel v6 for MDMLPPatch (3x3 unfold + per-channel linear 9->64).

out[n,c,p,e] = sum_d patches[n,c,p,d] * W[d,e] + b[e]
x: [16,64,56,56] f32, W: [9,64] f32, b: [64] f32 -> out: [16,64,3136,64] f32

Data-parallel over batch N: 16 n / 8 cores = 2 n per core -> 128 images/core.

Block-diagonal-weight scheme, full-width M=128 matmuls + batched tails:
  - K = 73 = 9 taps x 8 pixel-phases + ONE shared ones row (bias needs only
    one: w2[72, 64j+e] = b[e] for every j).  Host ships, per image,
    S2[8d+j, col] = patches[d, 8*col + j] in bf16 ([73, 392]); and
    w2[8d+j', 64j+e] = W[d,e]*(j==j') in bf16 ([73, 512]).
  - Image body = 3 chunks of 1024 pixels: one matmul per chunk
    (lhsT = S2[:, 128b:128b+128], rhs = w2) fills PSUM [128, 512] where
    partition q = pixels (1024b+8q .. +7) x 64ch = one contiguous DRAM run.
  - The 64-px tails of 16 images batch into ONE matmul [128, 512] via a
    3-dim lhsT AP (partition P = 8i+q <-> image i tail partition q); its
    out-DMA uses a 2-level partition decomposition [[200704,16],[512,8]].
  - Per image: 3 matmuls, one [128, 1536] DVE/ACT cast-copy (f32->bf16),
    one out-DMA.  Per 16-image block: 1 load, 1 tail matmul/copy/DMA.
  - Output is written bf16 (rel-err ~4e-3 << 2e-2 gate); host casts to f32.
"""

import numpy as np
import ml_dtypes

import concourse.bass as bass
import concourse.mybir as mybir
from concourse import bacc
from concourse.tile import TileContext
from concourse.bass_utils import run_bass_kernel_spmd

F32 = mybir.dt.float32
BF16 = mybir.dt.bfloat16
NP_BF16 = ml_dtypes.bfloat16

N_CORES = 8
IMGS = 128            # images per core (2 n x 64 c)
NPIX = 56 * 56        # 3136
KDIM = 10             # 9 taps + ones (bias) row
G = 8                 # pixels per partition-run
NCOL = NPIX // G      # 392
K2 = 9 * G + 1        # 73: 72 tap-phase rows + ONE ones row
KT = 9 * G            # 72
N2 = G * 64           # 512
IMG64 = NPIX * 64     # 200704 elements per image


def build_nc(imgs=IMGS, blk=16, psum_bufs=2, sh_bufs=2, stage_bufs=4,
             do_mm=True, do_copy=True, do_out=True, repeat=1,
             in_eng="scalar", out_eng="sync", out_bf16=True, pool_copy=0):
    assert imgs % blk == 0
    ODT = BF16 if out_bf16 else F32
    nc = bacc.Bacc("TRN2", target_bir_lowering=False, debug=False)
    sd = nc.dram_tensor("s", [K2, imgs, NCOL], BF16, kind="ExternalInput")
    wd = nc.dram_tensor("w", [K2, N2], BF16, kind="ExternalInput")
    out = nc.dram_tensor("out", [imgs * IMG64], ODT, kind="ExternalOutput")

    with TileContext(nc) as tc:
        with (
            tc.tile_pool(name="const", bufs=1) as constp,
            tc.tile_pool(name="shift", bufs=sh_bufs) as shiftp,
            tc.tile_pool(name="stage", bufs=stage_bufs) as stagep,
            tc.tile_pool(name="tstage", bufs=2) as tstagep,
            tc.tile_pool(name="tgat", bufs=2) as tgatp,
            tc.tile_pool(name="psum", bufs=psum_bufs, space="PSUM") as psump,
            tc.tile_pool(name="ptail", bufs=2, space="PSUM") as ptailp,
        ):
            wt = constp.tile([K2, N2], BF16)
            nc.sync.dma_start(out=wt[:, :], in_=wd[:, :])
            if not do_out:
                dummyt = constp.tile([K2, N2], ODT)
                nc.vector.tensor_copy(dummyt[:, :], wt[:, :])
                dummy = bass.AP(out, 0, [[N2, K2], [1, N2]])
                nc.sync.dma_start(out=dummy, in_=dummyt[:, :])

            in_dma = getattr(nc, in_eng).dma_start
            out_dma = getattr(nc, out_eng).dma_start
            copy_idx = 0

            def do_one_copy(dst, src):
                nonlocal copy_idx
                if pool_copy and copy_idx % pool_copy == pool_copy - 1:
                    nc.gpsimd.tensor_copy(dst, src)
                elif copy_idx % 2 == 0:
                    nc.vector.tensor_copy(dst, src)
                else:
                    nc.scalar.copy(dst, src)
                copy_idx += 1

            sh = None
            for it in range(imgs * repeat):
                img = it % imgs
                ib = img % blk
                if ib == 0:
                    sh = shiftp.tile([K2, blk * NCOL], BF16, tag="sh")
                    in_dma(out=sh[:, :], in_=sd[:, img:img + blk, :])
                    if do_mm:
                        # one batched tail matmul for the whole block:
                        # gather 16 images' tail cols -> contiguous [80, 128]
                        # (walrus rejects multi-free-dim matmul weights, so
                        # go through a tiny DVE gather-copy first);
                        # lhsT col (i, q) -> PSUM partition 8i+q
                        shap = sh[:, :]
                        src = bass.AP(
                            shap.tensor, shap.offset + (NCOL - G),
                            [list(shap.ap[0]), [NCOL, blk], [1, G]],
                        )
                        tg = tgatp.tile([K2, blk * G], BF16, tag="tg")
                        nc.vector.tensor_copy(tg[:, :], src)
                        pt = ptailp.tile([128, N2], F32, tag="pt")
                        nc.tensor.matmul(out=pt[:, :], lhsT=tg[:, :],
                                         rhs=wt[:, :], start=True, stop=True)
                        if do_copy:
                            ts = tstagep.tile([128, N2], ODT, tag="ts")
                            do_one_copy(ts[:, :], pt[:, :])
                            if do_out:
                                tail_ap = bass.AP(
                                    out, img * IMG64 + (NPIX - 64) * 64,
                                    [[IMG64, blk], [N2, G], [1, N2]],
                                )
                                out_dma(out=tail_ap, in_=ts[:, :])
                stage = stagep.tile([128, 3 * N2], ODT, tag="stage")
                if do_mm:
                    p = psump.tile([128, 3 * N2], F32, tag="p")
                    for b in range(3):
                        nc.tensor.matmul(
                            out=p[:, N2 * b:N2 * (b + 1)],
                            lhsT=sh[:, ib * NCOL + 128 * b:
                                    ib * NCOL + 128 * (b + 1)],
                            rhs=wt[:, :], start=True, stop=True,
                        )
                    if do_copy:
                        do_one_copy(stage[:, :], p[:, :])
                if not do_out:
                    continue
                out_ap = bass.AP(
                    out, img * IMG64,
                    [[N2, 128], [1024 * 64, 3], [1, N2]],
                )
                out_dma(out=out_ap, in_=stage[:, :])
    nc.compile()
    return nc


_CACHE = {}


def _get_nc(**kw):
    key = tuple(sorted(kw.items()))
    if key not in _CACHE:
        _CACHE[key] = build_nc(**kw)
    return _CACHE[key]


def _prep_inputs(x, W, b):
    x = np.ascontiguousarray(np.asarray(x, dtype=np.float32))
    W = np.ascontiguousarray(np.asarray(W, dtype=np.float32))
    b = np.ascontiguousarray(np.asarray(b, dtype=np.float32))
    N, C, H, Wd = x.shape
    nimg = N * C
    xpad = np.zeros((nimg, 58, 58), dtype=np.float32)
    xpad[:, 1:57, 1:57] = x.reshape(nimg, H, Wd)
    # S[img, d, p] = xpad[img, p//56 + d//3, p%56 + d%3]
    S = np.empty((nimg, 9, NPIX), dtype=np.float32)
    for d in range(9):
        di, dj = divmod(d, 3)
        S[:, d, :] = xpad[:, di:di + 56, dj:dj + 56].reshape(nimg, NPIX)
    # S2[img, 8d+j, col] = S[img, d, 8*col + j]; row 72 = shared ones row
    # (bias needs only ONE ones row: w2[72, 64j+e] = b[e] for every j)
    S2 = S.reshape(nimg, 9, NCOL, G).transpose(0, 1, 3, 2)
    S2 = np.concatenate(
        [S2.reshape(nimg, KT, NCOL), np.ones((nimg, 1, NCOL), np.float32)],
        axis=1,
    )
    # per-core [K2, IMGS, NCOL] so 16-image loads are 12.5 KB/descriptor
    S2T = np.ascontiguousarray(
        S2.reshape(N_CORES, IMGS, K2, NCOL).transpose(0, 2, 1, 3)
    ).astype(NP_BF16)
    w2 = np.zeros((9, G, G, 64), dtype=np.float32)
    for j in range(G):
        w2[:, j, j, :] = W
    w2 = np.concatenate(
        [w2.reshape(KT, N2), np.tile(b, G)[None, :]], axis=0)
    w2 = np.ascontiguousarray(w2).astype(NP_BF16)
    in_maps = [{"s": S2T[i], "w": w2} for i in range(N_CORES)]
    return in_maps, N, C


def run(x, W, b, trace=False, **kw):
    in_maps, N, C = _prep_inputs(x, W, b)
    nc = _get_nc()
    res = run_bass_kernel_spmd(
        nc, in_maps, core_ids=list(range(N_CORES)), trace=trace, **kw
    )
    outs = [
        res.results[i]["out"].astype(np.float32).reshape(
            N // N_CORES, C, NPIX, 64)
        for i in range(N_CORES)
    ]
    full = np.concatenate(outs, axis=0)
    return full, res


def kernel(x, W, b):
    full, _ = run(x, W, b, trace=False)
    return full
